# revision 29
# baseline (speedup 1.0000x reference)
"""ExaoneMoESparseMoEBlock Trainium2 kernel.

Strategy (expert-parallel over 8 NeuronCores):
  - Routing (gate matmul + biased grouped top-k) computed host-side in float64
    (selection margins >> fp32 noise, matches the fp32 jax reference).
  - Tokens are dispatched host-side. Experts are assigned to (core, slot) by
    token-count rank in a serpentine order so that every core's slot s holds a
    similarly-loaded expert: slot capacities are uniform across cores (SPMD)
    and per-core work is balanced.
  - Routed expert weights are stored in HBM as float8e3 (e3m4, x32 scale):
    4-bit mantissa on sigma=0.02 weights gives ~1.5e-2 final relative error
    (vs the 2e-2 gate), and halves the weight-stream bytes — the DMA floor.
    The PE consumes fp8 stationary x fp16 moving directly; the x32 scale is
    folded into the SiLU input scale (1/32) and the down-projection output
    scale (1/1024), both free. The shared expert stays fp16.
  - Each core runs the SiLU-gated MLP for its 8 experts in [feature, token]
    layout — weight tiles stationary, tokens moving at exact slot widths, so
    the PE streams every matmul at N=cap with LDWEIGHTS fully hidden.
  - Per expert, gate/up run i-chunk-blocked (gate-it, up-it, silu, mul) so
    only ~4 PSUM banks are live and activation drains hide under matmuls.
  - DMA issue is split by stream to avoid HWDGE FIFO head-of-line blocking:
    weights on the scalar queue, activation inputs on sync, outputs on the
    gpsimd SWDGE queue (final expert's outputs on sync so the SWDGE drain
    overlaps compute). All DRAM tensors are packed partition-major on host
    so every transfer is contiguous per partition (cheap descriptors);
    gate/up weights are packed i-major so expert 0's first i-slices land
    early and the cold-start supply stall is short.
  - Phase order: expert 0, then the shared expert (gate/up in two T=512
    halves, 4 PSUM banks each), with shared-down interleaved into expert 1's
    gate/up at i-chunk granularity so the strict-FIFO DVE queue never
    head-of-line blocks PSUM recycling; then experts 2..7.
  - The shared expert (IS=2048) is tensor-parallel sharded over the 8 cores
    (256 inter-dim slice each); each core emits a full [H, T] partial.
  - Host applies routing weights, scatter-adds expert outputs, and sums the
    shared partials.
"""

import sys
import types

import ml_dtypes
import numpy as np

F8NP = ml_dtypes.float8_e3m4

T, H, E, K_TOP = 1024, 2048, 64, 8
G, TG = 8, 4
I_DIM, IS_DIM = 1024, 2048
SCALE = 2.5
N_CORES = 8
EPC = E // N_CORES       # experts per core
ISC = IS_DIM // N_CORES  # shared-expert intermediate slice per core
HC = H // 128            # 16 h-chunks
IC = I_DIM // 128        # 8 i-chunks
CMAX = 512               # hard per-expert capacity limit (moving-dim max)
WS = 32.0                # fp8e3 weight-quantization scale (power of 2)

_LAST_RESULT = None      # BassKernelResults of the most recent run (for test.py)


def _install_ntff_shim():
    """Register the axon NTFF profile hook if the image's antenv lacks it.

    Lets BASS_TRACE=1 produce a perfetto trace + exec_time_ns. Harmless no-op
    when tracing is off or the axon .so is absent.
    """
    try:
        import antenv
        if "antenv.axon_hooks" in sys.modules:
            return
        mod = types.ModuleType("antenv.axon_hooks")
        mod._hook = None
        mod.set_axon_ntff_profile_hook = lambda h: setattr(mod, "_hook", h)
        mod.get_axon_ntff_profile_hook = lambda: mod._hook
        sys.modules["antenv.axon_hooks"] = mod
        antenv.axon_hooks = mod
        from trn_agent_boot.trn_boot import _ntff_profile_via_ctypes
        mod.set_axon_ntff_profile_hook(
            _ntff_profile_via_ctypes("/opt/axon/libaxon_pjrt.so")
        )
    except Exception:
        pass


def _routing(x, gate_w, e_bias):
    """float64 replica of the reference's sigmoid biased grouped top-k."""
    logits = x.astype(np.float64) @ gate_w.astype(np.float64)
    scores = 1.0 / (1.0 + np.exp(-logits))
    sb = scores + e_bias.astype(np.float64)[None, :]
    gsz = E // G
    gs = sb.reshape(T, G, gsz)
    top2 = np.sort(gs, axis=-1)[:, :, -2:].sum(-1)
    gidx = np.argsort(-top2, axis=-1, kind="stable")[:, :TG]
    gmask = np.zeros((T, G), bool)
    gmask[np.arange(T)[:, None], gidx] = True
    masked = np.where(np.repeat(gmask, gsz, axis=1), sb, -np.inf)
    idx = np.argsort(-masked, axis=-1, kind="stable")[:, :K_TOP]
    w = np.take_along_axis(scores, idx, axis=1).astype(np.float32)
    w = w / w.sum(-1, keepdims=True)
    return (w * np.float32(SCALE)).astype(np.float32), idx.astype(np.int64)


_KERNEL_CACHE = {}


def _build_kernel(caps):
    """Per-core SPMD Bass program. caps[s] = token columns of expert slot s."""
    from concourse import bacc
    import concourse.mybir as mybir
    import concourse.tile as tile

    F32 = mybir.dt.float32
    F16 = mybir.dt.float16
    F8 = mybir.dt.float8e3
    ACT = mybir.ActivationFunctionType

    nc = bacc.Bacc("TRN2", target_bir_lowering=False, debug=False)

    slots = len(caps)
    xe_d = [nc.dram_tensor(f"xe{s}", [128, HC, caps[s]], F16,
                           kind="ExternalInput") for s in range(slots)]
    wg_d = nc.dram_tensor("wg", [slots, 2, 128, 4, HC // 2, 256], F8,
                          kind="ExternalInput")
    wu_d = nc.dram_tensor("wu", [slots, 2, 128, 4, HC // 2, 256], F8,
                          kind="ExternalInput")
    wd_d = nc.dram_tensor("wd", [slots, 2, 128, IC, H // 2], F8,
                          kind="ExternalInput")
    xt_d = nc.dram_tensor("xt", [HC // 2, 128, 2, T], F16, kind="ExternalInput")
    wsg_d = nc.dram_tensor("wsg", [2, 128, HC // 2, ISC], F16,
                          kind="ExternalInput")
    wsu_d = nc.dram_tensor("wsu", [2, 128, HC // 2, ISC], F16,
                          kind="ExternalInput")
    wsd_d = nc.dram_tensor("wsd", [128, ISC // 128, H], F16,
                          kind="ExternalInput")
    yr_d = [nc.dram_tensor(f"yr{s}", [128, HC, caps[s]], F16,
                           kind="ExternalOutput") for s in range(slots)]
    ys_d = nc.dram_tensor("ys", [4, 128, 4, T], F16, kind="ExternalOutput")

    # adaptive weight-pool depth: large token capacities grow the xe/sg/a/o
    # slots, so shrink the 8KB-slot weight pipeline to fit 192KB/partition
    capmax = max(caps)
    other_kb = (2 * (HC * capmax * 2) + 2 * max(IC * capmax * 2, 4096)
                + 2 * max(IC * capmax * 2, 4096)
                + 4 * max(HC // 2 * capmax * 2, 4096) + 8 * 4096
                + 640 * 2) / 1024.0
    wbufs = int(max(2, min(14, (192 - other_kb - 6) // 8)))

    with tile.TileContext(nc) as tc:
        with (
            tc.tile_pool(name="wpool", bufs=wbufs) as wpool,  # 8KB/part slots
            tc.tile_pool(name="xpool", bufs=2) as xpool,
            tc.tile_pool(name="sgpool", bufs=2) as sgpool,
            tc.tile_pool(name="apool", bufs=2) as apool,
            tc.tile_pool(name="opool", bufs=4) as opool,
            tc.tile_pool(name="xtpool", bufs=8) as xtpool,
            tc.tile_pool(name="warm", bufs=1) as warmpool,
            tc.tile_pool(name="pp", bufs=8, space="PSUM") as pp,
        ):
            # PE p-state warm-up: the HAM governor only unthrottles the PE
            # (1.2 -> 2.4 GHz) after ~3us of continuous matmul execution.
            # The first real matmul can't start until its weights+tokens
            # land (~9.5us); run junk matmuls on a zeroed scratch tile
            # during that DMA window so the real stream starts at full
            # clock instead of paying the ramp on slot-0's matmuls.
            scratch = warmpool.tile([128, 640], F16)
            nc.vector.memset(scratch[:], 0)
            # 8 junk tiles keep the psum pool rotation phase unchanged;
            # only the first 4 get matmuls (~2.1us of warm-up — the real
            # stream starts ~9.5us and continues the 3us busy window)
            for k in range(8):
                jps = pp.tile([128, 512], F32, name="junk", tag="ps")
                if k < 7:
                    nc.tensor.matmul(jps[:], scratch[:, 0:128],
                                     scratch[:, 128:640], start=True,
                                     stop=True)
            # DMA queue split: weights -> scalar HWDGE, activation inputs ->
            # sync HWDGE, outputs -> gpsimd SWDGE.  Each stream's issue order
            # matches its dependency order, so no FIFO head-of-line blocking.
            def emit_expert_gu(e, sliced_first=False, interleave=None):
                cap = caps[e]
                xe_t = xpool.tile([128, HC, cap], F16, tag="xe")
                sg_t = sgpool.tile([128, IC, cap], F16, tag="sg")
                a_t = apool.tile([128, IC, cap], F16, tag="a")
                wg_h = [wpool.tile([128, 4, HC // 2, 256], F8, tag="w",
                                   name=f"wg{hh}")
                        for hh in range(2)]
                wu_h = [wpool.tile([128, 4, HC // 2, 256], F8, tag="w",
                                   name=f"wu{hh}")
                        for hh in range(2)]
                if sliced_first:
                    nc.sync.dma_start(xe_t[:, 0:6, :], xe_d[e].ap()[:, 0:6, :])
                    nc.sync.dma_start(xe_t[:, 6:11, :],
                                      xe_d[e].ap()[:, 6:11, :])
                    nc.sync.dma_start(xe_t[:, 11:HC, :],
                                      xe_d[e].ap()[:, 11:HC, :])
                    # wg rides scalar, wu rides sync (idle until the gated
                    # xt waves), one 256KB piece per i-group so completion
                    # semaphores land just ahead of each chunk's chains;
                    # i-groups 2-3 are issued later from stage0 to keep at
                    # most 4 issues outstanding per queue.
                    for q in (0, 1):
                        for t_l, t_d, eng in ((wg_h, wg_d, nc.scalar),
                                              (wu_h, wu_d, nc.sync)):
                            for hh in range(2):
                                eng.dma_start(t_l[hh][:, q:q + 1],
                                              t_d.ap()[e][hh][:, q:q + 1])
                    first_w[:] = [wg_h, wu_h]
                else:
                    nc.sync.dma_start(xe_t[:], xe_d[e].ap())
                    for hh in range(2):
                        nc.scalar.dma_start(wg_h[hh][:], wg_d.ap()[e][hh])
                        nc.scalar.dma_start(wu_h[hh][:], wu_d.ap()[e][hh])
                def mm_chain(w_h, it):
                    ps = pp.tile([128, cap], F32, name="ps", tag="ps")
                    for h in range(HC):
                        nc.tensor.matmul(
                            ps[:],
                            w_h[h // 8][:, it // 2, h % 8,
                                        (it % 2) * 128:(it % 2) * 128 + 128],
                            xe_t[:, h, :],
                            start=(h == 0), stop=(h == HC - 1),
                        )
                    return ps
                def drain(it, psg, psu):
                    nc.scalar.activation(sg_t[:, it, :], psg[:], ACT.Silu,
                                         scale=1.0 / WS)
                    nc.vector.tensor_mul(
                        a_t[:, it, :], sg_t[:, it, :], psu[:])
                def gu_chunk(it):
                    psg = mm_chain(wg_h, it)
                    psu = mm_chain(wu_h, it)
                    drain(it, psg, psu)
                if interleave is None:
                    for it in range(IC):
                        gu_chunk(it)
                elif sliced_first:
                    # both gate chains before the first up chain: covers the
                    # up-weight DMA with an extra 1.3us of matmuls
                    pg0 = mm_chain(wg_h, 0)
                    pg1 = mm_chain(wg_h, 1)
                    pu0 = mm_chain(wu_h, 0)
                    drain(0, pg0, pu0)
                    pu1 = mm_chain(wu_h, 1)
                    drain(1, pg1, pu1)
                    interleave(0)
                    for j in range(1, IC // 2):
                        gu_chunk(2 * j)
                        gu_chunk(2 * j + 1)
                        interleave(j)
                else:
                    for j in range(IC // 2):
                        gu_chunk(2 * j)
                        gu_chunk(2 * j + 1)
                        interleave(j)
                return a_t

            def emit_expert_down(e, a_t, wd_pre=None):
                cap = caps[e]
                for hh in range(2):
                    yo_e = opool.tile([128, HC // 2, cap], F16, tag="o")
                    if wd_pre is not None:
                        wd_t = wd_pre[hh]
                    else:
                        # last expert's wd rides sync (idle by then); the
                        # scalar row still carries the e7 gate/up stream
                        wd_t = wpool.tile([128, IC, 1024], F8, tag="w")
                        eng = nc.sync if e == slots - 1 else nc.scalar
                        eng.dma_start(wd_t[:], wd_d.ap()[e][hh])
                    for ht in range(IC):
                        psy = pp.tile([128, cap], F32, name="ps", tag="ps")
                        for ic in range(IC):
                            nc.tensor.matmul(
                                psy[:],
                                wd_t[:, ic, ht * 128:(ht + 1) * 128],
                                a_t[:, ic, :],
                                start=(ic == 0), stop=(ic == IC - 1),
                            )
                        nc.vector.tensor_scalar_mul(
                            yo_e[:, ht, :], psy[:], 1.0 / (WS * WS))
                    if e == slots - 1 and hh == 1:
                        # final drain: alternate sync/scalar so the last two
                        # pieces flush in parallel instead of serially
                        for q, eng in enumerate((nc.sync, nc.scalar,
                                                 nc.sync, nc.scalar)):
                            eng.dma_start(
                                yr_d[e].ap()[:, 8 + 2 * q:10 + 2 * q, :],
                                yo_e[:, 2 * q:2 * q + 2, :])
                    elif e == slots - 1:
                        nc.sync.dma_start(
                            yr_d[e].ap()[:, hh * 8:(hh + 1) * 8, :],
                            yo_e[:])
                    else:
                        nc.gpsimd.dma_start(
                            yr_d[e].ap()[:, hh * 8:(hh + 1) * 8, :],
                            yo_e[:])

            def emit_shared_gu(xt_ts):
                # gate/up in two T=512 halves: each half uses 4 PSUM banks and
                # drains (silu+mul) under the other half's matmuls.
                wsg_t = wpool.tile([128, HC, ISC], F16, tag="w")
                nc.scalar.dma_start(wsg_t[:, 0:HC // 2, :], wsg_d.ap()[0])
                nc.scalar.dma_start(wsg_t[:, HC // 2:HC, :], wsg_d.ap()[1])
                wsu_t = wpool.tile([128, HC, ISC], F16, tag="w")
                nc.scalar.dma_start(wsu_t[:, 0:HC // 2, :], wsu_d.ap()[0])
                nc.scalar.dma_start(wsu_t[:, HC // 2:HC, :], wsu_d.ap()[1])
                wsd_t = wpool.tile([128, ISC // 128, H], F16, tag="w")
                nc.scalar.dma_start(wsd_t[:], wsd_d.ap())
                sg_s = sgpool.tile([128, 2, T], F16, tag="sg")
                sa_s = apool.tile([128, 2, T], F16, tag="a")
                for th in range(2):
                    tsl = slice(th * 512, (th + 1) * 512)
                    psg2 = [pp.tile([128, 512], F32, name="ps", tag="ps")
                            for _ in range(2)]
                    psu2 = [pp.tile([128, 512], F32, name="ps", tag="ps")
                            for _ in range(2)]
                    for hc in range(HC):
                        xt_t = xt_ts[hc // 2][:, hc % 2, tsl]
                        for it in range(2):
                            nc.tensor.matmul(
                                psg2[it][:],
                                wsg_t[:, hc, it * 128:(it + 1) * 128],
                                xt_t,
                                start=(hc == 0), stop=(hc == HC - 1),
                            )
                            nc.tensor.matmul(
                                psu2[it][:],
                                wsu_t[:, hc, it * 128:(it + 1) * 128],
                                xt_t,
                                start=(hc == 0), stop=(hc == HC - 1),
                            )
                    for it in range(2):
                        nc.scalar.activation(
                            sg_s[:, it, tsl], psg2[it][:], ACT.Silu)
                        nc.vector.tensor_mul(
                            sa_s[:, it, tsl], sg_s[:, it, tsl], psu2[it][:])
                return sa_s, wsd_t

            def emit_shared_down(sa_s, wsd_t, htgs):
                for htg in htgs:
                    yo = opool.tile([128, 2, T], F16, tag="o")
                    for hi in range(2):
                        ht = htg * 2 + hi
                        psy_s = [pp.tile([128, 512], F32, name="ps", tag="ps")
                                 for _ in range(2)]
                        for ic in range(2):
                            for nh in range(2):
                                nc.tensor.matmul(
                                    psy_s[nh][:],
                                    wsd_t[:, ic, ht * 128:(ht + 1) * 128],
                                    sa_s[:, ic, nh * 512:(nh + 1) * 512],
                                    start=(ic == 0), stop=(ic == 1),
                                )
                        nc.vector.tensor_copy(yo[:, hi, 0:512], psy_s[0][:])
                        nc.vector.tensor_copy(yo[:, hi, 512:1024], psy_s[1][:])
                    nc.gpsimd.dma_start(
                        ys_d.ap()[htg // 2][:, (htg % 2) * 2:(htg % 2) * 2 + 2, :],
                        yo[:])

            xt_ts = []
            wd_pre = []
            first_w = []

            XT_WAVES = {3: (0, 1)}

            def stage0(j):
                # expert-0's gate/up i-groups 2-3 stream from here so their
                # completion semaphores land just ahead of each chunk's
                # chains with at most 4 issues outstanding per queue
                if j in (0, 1):
                    q = j + 2
                    wg_h, wu_h = first_w
                    for hh in range(2):
                        nc.scalar.dma_start(wg_h[hh][:, q:q + 1],
                                            wg_d.ap()[0][hh][:, q:q + 1])
                    for hh in range(2):
                        nc.sync.dma_start(wu_h[hh][:, q:q + 1],
                                          wu_d.ap()[0][hh][:, q:q + 1])
                # wd0 preloads ride sync (a pure-DMA queue): a 1MB issue on
                # scalar ring-stalls the queue and delays the silu stream,
                # which is what recycles the gate/up psum banks
                if j in (1, 2):
                    hh = j - 1
                    wd_t = wpool.tile([128, IC, 1024], F8, tag="w",
                                      name=f"wd0_{hh}")
                    nc.sync.dma_start(wd_t[:], wd_d.ap()[0][hh])
                    wd_pre.append(wd_t)
                # xt waves follow wd0 on the in-order sync queue, so they
                # can't crowd out anything the routed path still needs
                for hb in XT_WAVES.get(j, ()):
                    xt_t = xtpool.tile([128, 2, T], F16, name=f"xt{hb}", tag="xt")
                    nc.sync.dma_start(xt_t[:], xt_d.ap()[hb])
                    xt_ts.append(xt_t)

            a0 = emit_expert_gu(0, sliced_first=True, interleave=stage0)
            emit_expert_down(0, a0, wd_pre=wd_pre)
            # remaining xt waves flow gate-free during expert-0's down
            # phase: HBM is quiet there (wd0 preloaded, outputs via SWDGE)
            for hb in (2, 3, 4, 5, 6, 7):
                xt_t = xtpool.tile([128, 2, T], F16, name=f"xt{hb}", tag="xt")
                nc.sync.dma_start(xt_t[:], xt_d.ap()[hb])
                xt_ts.append(xt_t)
            sa_s, wsd_t = emit_shared_gu(xt_ts)
            a1 = emit_expert_gu(1, interleave=lambda j: emit_shared_down(
                sa_s, wsd_t, [2 * j, 2 * j + 1]))
            emit_expert_down(1, a1)
            for e in range(2, slots):
                a_t = emit_expert_gu(e)
                emit_expert_down(e, a_t)

    nc.compile()
    return nc


def kernel(hidden_states, gate_w, e_bias, w_gate, w_up, w_down,
           ws_gate, ws_up, ws_down):
    global _LAST_RESULT
    _install_ntff_shim()
    from concourse.bass_utils import run_bass_kernel_spmd

    x = np.ascontiguousarray(np.asarray(hidden_states, dtype=np.float32))
    gate_w = np.asarray(gate_w, dtype=np.float32)
    e_bias = np.asarray(e_bias, dtype=np.float32)
    w_gate = np.ascontiguousarray(np.asarray(w_gate, dtype=np.float32))
    w_up = np.ascontiguousarray(np.asarray(w_up, dtype=np.float32))
    w_down = np.ascontiguousarray(np.asarray(w_down, dtype=np.float32))
    ws_gate = np.ascontiguousarray(np.asarray(ws_gate, dtype=np.float32))
    ws_up = np.ascontiguousarray(np.asarray(ws_up, dtype=np.float32))
    ws_down = np.ascontiguousarray(np.asarray(ws_down, dtype=np.float32))

    w_route, idx = _routing(x, gate_w, e_bias)

    # per-expert token lists + per-token routing weights; experts with more
    # than CMAX tokens are split into multiple shards, empty experts dropped
    shards = []  # (expert_id, token_ids, weights)
    for e in range(E):
        te = np.nonzero((idx == e).any(axis=1))[0]
        if len(te) == 0:
            continue
        k_of_t = (idx[te] == e).argmax(axis=1)
        we = w_route[te, k_of_t]
        for s0 in range(0, len(te), CMAX):
            shards.append((e, te[s0:s0 + CMAX], we[s0:s0 + CMAX]))
    while len(shards) % N_CORES != 0:
        shards.append((0, np.zeros(0, np.int64), np.zeros(0, np.float32)))
    n_slots = len(shards) // N_CORES

    # serpentine count-ranked assignment: slot s of core c gets shard
    # perm[c][s]; slot capacities are uniform across cores.
    scounts = np.array([len(s[1]) for s in shards])
    order = np.argsort(-scounts, kind="stable")
    perm = np.zeros((N_CORES, n_slots), np.int64)
    for s in range(n_slots):
        grp = order[s * N_CORES:(s + 1) * N_CORES]
        perm[:, s] = grp if s % 2 == 0 else grp[::-1]
    caps = tuple(
        int(max(4, scounts[perm[:, s]].max()))
        for s in range(n_slots)
    )

    if caps not in _KERNEL_CACHE:
        _KERNEL_CACHE[caps] = _build_kernel(caps)
    nc = _KERNEL_CACHE[caps]

    x16 = x.astype(np.float16)
    xt_l = np.ascontiguousarray(
        x16.T.reshape(8, 2, 128, T).transpose(0, 2, 1, 3))
    wg_q = (w_gate * np.float32(WS)).astype(F8NP)
    wu_q = (w_up * np.float32(WS)).astype(F8NP)
    wd_q = (w_down * np.float32(WS)).astype(F8NP)
    in_maps = []
    for c in range(N_CORES):
        sh = [shards[j] for j in perm[c]]
        es = np.array([s[0] for s in sh])
        in_map = {"xt": xt_l}
        for s in range(n_slots):
            te = sh[s][1]
            buf = np.zeros((caps[s], H), np.float16)
            buf[: len(te)] = x16[te]
            # [cap, H] -> [H, cap] -> [HC, 128, cap] -> partition-major
            in_map[f"xe{s}"] = np.ascontiguousarray(
                buf.T.reshape(HC, 128, caps[s]).transpose(1, 0, 2))
        in_map["wg"] = np.ascontiguousarray(
            wg_q[es].reshape(n_slots, 2, 8, 128, 4, 256)
            .transpose(0, 1, 3, 4, 2, 5))
        in_map["wu"] = np.ascontiguousarray(
            wu_q[es].reshape(n_slots, 2, 8, 128, 4, 256)
            .transpose(0, 1, 3, 4, 2, 5))
        in_map["wd"] = np.ascontiguousarray(
            wd_q[es].reshape(n_slots, IC, 128, 2, H // 2)
            .transpose(0, 3, 2, 1, 4))
        in_map["wsg"] = np.ascontiguousarray(
            ws_gate[:, c * ISC:(c + 1) * ISC].astype(np.float16)
            .reshape(2, 8, 128, ISC).transpose(0, 2, 1, 3))
        in_map["wsu"] = np.ascontiguousarray(
            ws_up[:, c * ISC:(c + 1) * ISC].astype(np.float16)
            .reshape(2, 8, 128, ISC).transpose(0, 2, 1, 3))
        in_map["wsd"] = np.ascontiguousarray(
            ws_down[c * ISC:(c + 1) * ISC].astype(np.float16)
            .reshape(2, 128, H).transpose(1, 0, 2))
        in_maps.append(in_map)

    try:
        res = run_bass_kernel_spmd(nc, in_maps,
                                   core_ids=list(range(N_CORES)))
    except Exception:
        res = run_bass_kernel_spmd(nc, in_maps,
                                   core_ids=list(range(N_CORES)))
    _LAST_RESULT = res

    y = np.zeros((H, T), np.float32)
    for c in range(N_CORES):
        y += (res.results[c]["ys"].transpose(0, 2, 1, 3)
              .reshape(H, T).astype(np.float32))
    out = np.ascontiguousarray(y.T)
    for c in range(N_CORES):
        for s in range(n_slots):
            _, te, we = shards[perm[c][s]]
            cnt = len(te)
            if cnt == 0:
                continue
            yr = res.results[c][f"yr{s}"].astype(np.float32)
            O = yr.transpose(1, 0, 2).reshape(H, caps[s])[:, :cnt]
            out[te] += we[:, None] * O.T
    return out

